# revision 8
# baseline (speedup 1.0000x reference)
"""Trainium2 Bass kernel for nn_MultiHeadAttention_18425409700485.

B=2, S=2048, D=1024, H=16 heads (DH=64). 8 NeuronCores:
core c handles batch b = c // 4 and head group hg = c % 4 (4 heads each).

Reference semantics (note the deliberate quirks faithfully reproduced):
  q = query @ Wq ; k = key @ Wk ; v = value @ Wv           (biases are zero)
  scores = q k^T per head; causal mask of -1e9 added BEFORE dividing by
  sqrt(D)=32; softmax; x = attn @ v  [B,H,S,DH]
  "buggy" merge: x.swapaxes(-1,-2).reshape(B,-1,D) -> merged rows
  R = h*128 + 2*dh + t hold x[t*1024 + c, dh] at column c.
  out = merged @ Wo.

Per-core dataflow (all matmul contractions need the contracted dim on
partitions, so inputs are transposed on-chip via one SBUF->SBUF xbar
DMA-transpose per input after a casting fp32->bf16 SWDGE load):
  xT[p, a, b, f] = X^T[128b+p, 128a+f]    (one dma_start_transpose call)
  qT/kT [128(2 heads x 64), pair, S]  via Wq/Wk as stationary
  v65   [128(s), 16(s-tile), 4*65]    v natural + ones column per head
  scoresT tiles [j(128), i(512)] = K Q^T ; exp on ACT (scale=1/32);
  causal handled by multiplying diagonal tiles with a 0/1 mask (bf16);
  x_unnorm^T [65, 512] accumulated with lhsT=[v|1] (row 64 = softmax denom);
  PE-transpose chunks -> x natural, scale by reciprocal of denom;
  output projection with lhsT = interleaved x tiles (the buggy merge is a
  free-dim access pattern), rhs = Wo chunks; rows DMA'd out contiguously.
"""

import os
import sys

sys.path.insert(0, "/opt/trn_rl_repo")

import numpy as np

S = 2048
D = 1024
H_PER_CORE = 4
DH = 64
NEG = -1.0e9
SCALE = 1.0 / 32.0  # 1/sqrt(D)

_CACHE = {}


def _build_kernel():
    import concourse.bass as bass
    import concourse.mybir as mybir
    import concourse.tile as tile
    from concourse import bacc
    from concourse.masks import make_identity

    fp32 = mybir.dt.float32
    bf16 = mybir.dt.bfloat16

    nc = bacc.Bacc("TRN2", target_bir_lowering=False, debug=False,
                   enable_asserts=False)

    xq = nc.dram_tensor("xq", [S, D], fp32, kind="ExternalInput").ap()
    xk = nc.dram_tensor("xk", [S, D], fp32, kind="ExternalInput").ap()
    xv = nc.dram_tensor("xv", [S, D], fp32, kind="ExternalInput").ap()
    wq = nc.dram_tensor("wq", [D, 256], fp32, kind="ExternalInput").ap()
    wk = nc.dram_tensor("wk", [D, 256], fp32, kind="ExternalInput").ap()
    wv = nc.dram_tensor("wv", [D, 256], fp32, kind="ExternalInput").ap()
    wo = nc.dram_tensor("wo", [D, D], fp32, kind="ExternalInput").ap()
    out = nc.dram_tensor("out", [512, D], fp32, kind="ExternalOutput").ap()

    Exp = mybir.ActivationFunctionType.Exp

    with tile.TileContext(nc) as tc:
        from contextlib import ExitStack

        with ExitStack() as ctx:
            const = ctx.enter_context(tc.tile_pool(name="const", bufs=1))
            persist = ctx.enter_context(tc.tile_pool(name="persist", bufs=1))

            # --- constants -------------------------------------------------
            ident = const.tile([128, 128], bf16, name="ident")
            make_identity(nc, ident)

            # mask4[:, o, f] = 1.0 where f >= 128*o + p else 0.0
            mask4 = const.tile([128, 4, 512], bf16, name="mask4")
            nc.gpsimd.memset(mask4[:], 1.0)
            for o in range(4):
                nc.gpsimd.affine_select(
                    out=mask4[:, o, :],
                    in_=mask4[:, o, :],
                    compare_op=mybir.AluOpType.is_ge,
                    fill=0.0,
                    base=-128 * o,
                    pattern=[[1, 512]],
                    channel_multiplier=-1,
                )

            # --- weights (cast to bf16 during SWDGE DMA) -------------------
            wq_sb = const.tile([128, 8, 256], bf16, name="wq_sb")
            wk_sb = const.tile([128, 8, 256], bf16, name="wk_sb")
            wv_sb = const.tile([128, 8, 256], bf16, name="wv_sb")
            wo_sb = const.tile([128, 8, 1024], bf16, name="wo_sb")
            # weight loads are emitted inside phase A in dependency order so
            # the DMA queue feeds the PE as early as possible (wo goes last).

            # --- persistent activations -----------------------------------
            # qT/kT: [dh-part (2 heads x 64), pair, S]
            qT = persist.tile([128, 2, S], bf16, name="qT")
            kT = persist.tile([128, 2, S], bf16, name="kT")
            # v natural + ones col: per s-tile, 4 heads x 65 cols
            v65 = persist.tile([128, 16, 4 * 65], bf16, name="v65")
            nc.gpsimd.memset(
                v65.rearrange("p t (h c) -> p t h c", c=65)[:, :, :, 64], 1.0
            )
            # x per head, pre-interleaved for the buggy merge: the
            # output-projection stationary needs a single free dim, so store
            # xall[p, h, q, 2*d + t] = x[1024*t + 128*q + p, d].
            xall = persist.tile([128, H_PER_CORE, 8, 128], bf16, name="xall")

            # ==============================================================
            # Phase A: load + transpose inputs, projections
            # ==============================================================
            with ExitStack() as phase_a:
                stage = phase_a.enter_context(
                    tc.tile_pool(name="stage", bufs=2))
                xt_pool = phase_a.enter_context(
                    tc.tile_pool(name="xt_pool", bufs=2))
                ppsum = phase_a.enter_context(
                    tc.tile_pool(name="ppsum", bufs=2, space="PSUM"))

                def load_transposed(dram_ap, tag):
                    # two half-loads so the SWDGE cast-load of the next half
                    # (and next input) overlaps the xbar transpose.
                    xT = xt_pool.tile([128, 16, 8, 128], bf16, tag="xT",
                                      name=f"xT_{tag}")
                    for hf in range(2):
                        xnat = stage.tile([128, 8, D], bf16, tag="xnat",
                                          name=f"xnat_{tag}{hf}")
                        nc.gpsimd.dma_start(
                            xnat[:],
                            dram_ap[1024 * hf:1024 * (hf + 1), :].rearrange(
                                "(t p) d -> p t d", p=128))
                        # xT[p, a, b, f] = X[128a+f, 128b+p]
                        nc.sync.dma_start(
                            xT[:, 8 * hf:8 * (hf + 1), :, :].rearrange(
                                "p a b f -> p (a b) f"),
                            xnat.rearrange("p t d -> p (t d)"),
                            transpose=True,
                        )
                    return xT

                # ---- value -> v65 (natural layout, lhsT = xT_v chunks) ----
                nc.gpsimd.dma_start(wv_sb[:],
                                    wv.rearrange("(o p) m -> p o m", p=128))
                xT_v = load_transposed(xv, "v")
                for t in range(16):
                    ps = ppsum.tile([128, 512], fp32, tag="ppsum", name=f"psv_{t}")
                    for dc in range(8):
                        nc.tensor.matmul(
                            ps[:, :256],
                            lhsT=xT_v[:, t, dc, :],
                            rhs=wv_sb[:, dc, :],
                            start=(dc == 0),
                            stop=(dc == 7),
                        )
                    nc.vector.tensor_copy(
                        v65.rearrange("p t (h c) -> p t h c", c=65)[:, t, :, :64],
                        ps[:, :256].rearrange("p (h c) -> p h c", c=64),
                    )

                # ---- query/key -> qT/kT (transposed, lhsT = W chunks) -----
                nc.gpsimd.dma_start(wq_sb[:],
                                    wq.rearrange("(o p) m -> p o m", p=128))
                nc.gpsimd.dma_start(wk_sb[:],
                                    wk.rearrange("(o p) m -> p o m", p=128))
                for dram_ap, w_sb, dst, tag in (
                    (xq, wq_sb, qT, "q"),
                    (xk, wk_sb, kT, "k"),
                ):
                    xT = load_transposed(dram_ap, tag)
                    for a in range(2):
                        for ic in range(4):
                            ps = ppsum.tile([128, 512], fp32, tag="ppsum",
                                            name=f"ps_{tag}_{a}_{ic}")
                            for dc in range(8):
                                nc.tensor.matmul(
                                    ps[:],
                                    lhsT=w_sb[:, dc, 128 * a:128 * (a + 1)],
                                    rhs=xT[:, 4 * ic:4 * (ic + 1), dc, :],
                                    start=(dc == 0),
                                    stop=(dc == 7),
                                )
                            nc.vector.tensor_copy(
                                dst[:, a, 512 * ic:512 * (ic + 1)], ps[:])

            nc.gpsimd.dma_start(wo_sb[:],
                                wo.rearrange("(o p) m -> p o m", p=128))

            # ==============================================================
            # Phase B: attention + output projection, per head
            # ==============================================================
            with ExitStack() as phase_b:
                spsum = phase_b.enter_context(
                    tc.tile_pool(name="spsum", bufs=1, space="PSUM"))
                xpsum = phase_b.enter_context(
                    tc.tile_pool(name="xpsum", bufs=2, space="PSUM"))
                tpsum = phase_b.enter_context(
                    tc.tile_pool(name="tpsum", bufs=1, space="PSUM"))
                opsum = phase_b.enter_context(
                    tc.tile_pool(name="opsum", bufs=1, space="PSUM"))
                ptile = phase_b.enter_context(tc.tile_pool(name="ptile", bufs=4))
                misc = phase_b.enter_context(tc.tile_pool(name="misc", bufs=2))
                outp = phase_b.enter_context(tc.tile_pool(name="outp", bufs=2))

                for a in range(2):  # head pair; rows 0-63 / 64-127
                    for ic in range(4):
                        pxs = [xpsum.tile([128, 512], fp32, tag="px",
                                          name=f"px_{a}_{ic}_{sg}")
                               for sg in range(2)]
                        nlive = 4 * (ic + 1)
                        nbatch = nlive // 2
                        pbs = [[None] * 2 for _ in range(nbatch)]
                        # software-pipelined: scores+exp for batch b2, then
                        # attn@v matmuls for batch b2-1. The two heads of the
                        # pair use disjoint PE row groups (tile_position rows
                        # 0 / 64), so their scores matmuls run concurrently.
                        for b2 in range(nbatch + 1):
                            if b2 < nbatch:
                                pss = [spsum.tile([128, 1024], fp32,
                                                  tag=f"ps{sg}",
                                                  name=f"ps_{a}_{ic}_{b2}_{sg}")
                                       for sg in range(2)]
                                for k2 in range(2):
                                    jj = 2 * b2 + k2
                                    for sg in range(2):
                                        po = 64 * sg
                                        nc.tensor.matmul(
                                            pss[sg][:, 512 * k2:512 * (k2 + 1)],
                                            lhsT=kT[po:po + 64, a,
                                                    128 * jj:128 * (jj + 1)],
                                            rhs=qT[po:po + 64, a,
                                                   512 * ic:512 * (ic + 1)],
                                            start=True,
                                            stop=True,
                                        )
                                for sg in range(2):
                                    pb = ptile.tile([128, 2, 512], bf16,
                                                    tag="pb",
                                                    name=f"pb_{a}_{ic}_{b2}_{sg}")
                                    pb2d = pb.rearrange("p k f -> p (k f)")
                                    nc.scalar.activation(pb2d, pss[sg][:], Exp,
                                                         scale=SCALE)
                                    if 2 * b2 >= 4 * ic:  # diagonal: mask
                                        o0 = 2 * b2 - 4 * ic
                                        nc.vector.tensor_mul(
                                            pb2d, pb2d,
                                            mask4[:, o0:o0 + 2, :].rearrange(
                                                "p k f -> p (k f)"),
                                        )
                                    pbs[b2][sg] = pb
                            if b2 >= 1:
                                for k2 in range(2):
                                    jj = 2 * (b2 - 1) + k2
                                    for sg in range(2):
                                        h = 2 * a + sg
                                        nc.tensor.matmul(
                                            pxs[sg][:65, :],
                                            lhsT=v65[:, jj,
                                                     65 * h:65 * (h + 1)],
                                            rhs=pbs[b2 - 1][sg][:, k2, :],
                                            start=(jj == 0),
                                            stop=(jj == nlive - 1),
                                        )
                        # x fixup: transpose to natural + normalize
                        for sg in range(2):
                            h = 2 * a + sg
                            px = pxs[sg]
                            xt_sb = misc.tile([65, 512], bf16, tag="xt_sb",
                                              name=f"xt_{h}_{ic}")
                            nc.vector.tensor_copy(xt_sb[:], px[:65, :])
                            pt = tpsum.tile([128, 4, 66], bf16, tag="pt",
                                            name=f"pt_{h}_{ic}")
                            pt3 = pt[:, :, :65]
                            for k4 in range(4):
                                nc.tensor.transpose(
                                    pt3[:, k4, :],
                                    xt_sb[:, 128 * k4:128 * (k4 + 1)],
                                    ident[:65, :65],
                                )
                            recip4 = misc.tile([128, 4], fp32, tag="recip4",
                                               name=f"rc_{h}_{ic}")
                            nc.vector.reciprocal(recip4[:], pt3[:, :, 64])
                            for k4 in range(4):
                                j = 4 * ic + k4  # i-chunk: i = 128*j + p
                                nc.vector.tensor_scalar_mul(
                                    xall[:, h, j % 8, (j // 8)::2],
                                    pt3[:, k4, :64],
                                    recip4[:, k4:k4 + 1],
                                )
                    # ---- output projection for the pair's heads ----
                    for sg in range(2):
                        h = 2 * a + sg
                        ot = outp.tile([128, 2, 512], fp32, tag="ot",
                                       name=f"ot_{h}")
                        for nn in range(2):
                            po_ = opsum.tile([128, 512], fp32, tag="po",
                                             name=f"po_{h}_{nn}")
                            for q8 in range(8):
                                nc.tensor.matmul(
                                    po_[:],
                                    lhsT=xall[:, h, q8, :],
                                    rhs=wo_sb[:, q8, 512 * nn:512 * (nn + 1)],
                                    start=(q8 == 0),
                                    stop=(q8 == 7),
                                )
                            nc.vector.tensor_copy(ot[:, nn, :], po_[:])
                        nc.sync.dma_start(
                            out[128 * h:128 * (h + 1), :],
                            ot.rearrange("p k f -> p (k f)"),
                        )

    nc.compile()
    return nc


def _get_nc():
    if "nc" not in _CACHE:
        _CACHE["nc"] = _build_kernel()
    return _CACHE["nc"]


def kernel(query, key, value, Wq, bq, Wk, bk, Wv, bv, Wo, bo):
    """Full inputs in, full output out. Shards batch x head-group over 8 cores."""
    nc = _get_nc()
    from concourse.bass_utils import run_bass_kernel_spmd

    query = np.ascontiguousarray(np.asarray(query, dtype=np.float32))
    key = np.ascontiguousarray(np.asarray(key, dtype=np.float32))
    value = np.ascontiguousarray(np.asarray(value, dtype=np.float32))
    Wq = np.ascontiguousarray(np.asarray(Wq, dtype=np.float32))
    Wk = np.ascontiguousarray(np.asarray(Wk, dtype=np.float32))
    Wv = np.ascontiguousarray(np.asarray(Wv, dtype=np.float32))
    Wo = np.ascontiguousarray(np.asarray(Wo, dtype=np.float32))

    in_maps = []
    for c in range(8):
        b, hg = c // 4, c % 4
        cols = slice(256 * hg, 256 * (hg + 1))
        in_maps.append({
            "xq": query[b],
            "xk": key[b],
            "xv": value[b],
            "wq": np.ascontiguousarray(Wq[:, cols]),
            "wk": np.ascontiguousarray(Wk[:, cols]),
            "wv": np.ascontiguousarray(Wv[:, cols]),
            "wo": Wo,
        })

    trace = bool(int(os.environ.get("KERNEL_TRACE", "0")))
    res = run_bass_kernel_spmd(nc, in_maps, core_ids=list(range(8)),
                               trace=trace)
    _CACHE["last_result"] = res

    B = query.shape[0]
    full = np.zeros((B, S, D), dtype=np.float32)
    for c in range(8):
        b, hg = c // 4, c % 4
        full[b, 512 * hg:512 * (hg + 1), :] = res.results[c]["out"]
    return full
